# revision 21
# baseline (speedup 1.0000x reference)
"""Trainium2 Bass kernel (b4) for nn_DenseGraphConvEdgeToEdge (B=4, N=256, C=O=128).

out[b,i,j,:] = E[b,i,j]@W0 + E[b,j,i]@W1 + R[b,i]@W2 + Cm[b,j]@W3
             + R[b,j]@W4 + Cm[b,i]@W5 + sa[b]@W6 + bias
where R = E.sum(axis=2) (row sums), Cm = E.sum(axis=1) (col sums),
sa = E.sum(axis=(1,2)).

b4 column-pairing restructure (vs b3's 112us): core (b, h) owns the two
quadrants of COLUMN h of batch b: rtA = quad (1-h, h) ships first,
rtB = quad (h, h) second. Both output quads share one column block, so
the j-dependent broadcast tile G[j] = Cm[j]@W3 + R[j]@W4 is a SINGLE
tile per core, and its inputs are: Cm-block-h (fully local: my two cs),
my rs(rtB), and the sibling's rs(rtA) - which arrives in an EARLY
pairwise AllGather (ex1) fired as soon as rtA lands (~16us). The second
exchange (rs/cs of rtB) is only needed by P_rtA (drain-time of the
second quad) and sa, both hidden under the first quad's mains. Rank
asymmetry (which AllGather slot is the sibling) is handled by
host-zeroed weight variants in gwt/pwt - 128-row dead matmuls instead
of branches. Marginal packs are f16 end-to-end: DVE rowsum reduces run
in 2x_1p mode (all-16-bit operands), and the exchange payload needs no
cast hop. PE does colsums via identity-matmul. DMA: rt fp16, vt/W1
fp8e4, out fp8e4 centered by sa@W6+bias (shipped as a f32 row, added
back on host: per-channel zero-point dequant). Host-measured rel err of
the quantization scheme: 5.4e-3 (gate 2e-2).
"""
import numpy as np
import ml_dtypes

import concourse.mybir as mybir
import concourse.tile as tile
from concourse import bacc
from concourse.bass_utils import run_bass_kernel_spmd

F32 = mybir.dt.float32
F32R = mybir.dt.float32r
F16 = mybir.dt.float16
F8 = mybir.dt.float8e4
ADD = mybir.AluOpType.add
E_NP = np.float16
E8_NP = ml_dtypes.float8_e4m3

B, N, C, O = 4, 256, 128, 128
Q = 128
QF = Q * Q          # 16384
N_CORES = 8
NCHUNK = 8          # rt chunks (512KB; 4KB/partition descriptors)
CH = QF // NCHUNK   # 2048 cols per rt DMA chunk
NVCHUNK = 2         # fp8 vt chunks: 8KB/partition descriptors
VCH = QF // NVCHUNK
NG = 32             # 512-col psum groups per quadrant
STB = 4096          # staging width per output DMA (4KB/partition fp8)
PARK = 5
DVE_SHARE = 42      # of 64 drains on DVE (512-wide broadcast t_t), rest ACT

_NC_CACHE = {}


def _drain_on_dve(idx, total=2 * NG, share=DVE_SHARE):
    return (idx * share) // total != ((idx + 1) * share) // total


def build(use_collective=True):
    nc = bacc.Bacc(trn_type="TRN2")

    rtA_d = nc.dram_tensor("rtA", [C, QF], F16, kind="ExternalInput")
    rtB_d = nc.dram_tensor("rtB", [C, QF], F16, kind="ExternalInput")
    vtA_d = nc.dram_tensor("vtA", [C, QF], F8, kind="ExternalInput")
    vtB_d = nc.dram_tensor("vtB", [C, QF], F8, kind="ExternalInput")
    w0_d = nc.dram_tensor("w0m", [C, O], F16, kind="ExternalInput")
    w1_d = nc.dram_tensor("w1m", [C, O], F8, kind="ExternalInput")
    i2_d = nc.dram_tensor("i2", [C, 4 * Q], F16, kind="ExternalInput")
    gwt_d = nc.dram_tensor("gwt", [C, 4 * O], F16, kind="ExternalInput")
    pwt_d = nc.dram_tensor("pwt", [C, 6 * O], F16, kind="ExternalInput")
    wsb6_d = nc.dram_tensor("wsb6", [C, O], F32, kind="ExternalInput")
    biasr_d = nc.dram_tensor("biasr", [1, O], F32, kind="ExternalInput")
    outA = nc.dram_tensor("outA", [O, QF], F8, kind="ExternalOutput")
    outB = nc.dram_tensor("outB", [O, QF], F8, kind="ExternalOutput")
    sbrow_o = nc.dram_tensor("sbrow_o", [1, O], F32, kind="ExternalOutput")

    with tile.TileContext(nc) as tc:
        with (
            tc.tile_pool(name="pool", bufs=1) as pool,
            tc.tile_pool(name="stpool", bufs=3) as stpool,
            tc.tile_pool(name="ppmain", bufs=5, space="PSUM") as ppmain,
            tc.tile_pool(name="ppaux", bufs=3, space="PSUM") as ppaux,
            tc.tile_pool(name="dram", bufs=1, space="DRAM") as dram,
        ):
            # ---- small constants first ----
            i2t = pool.tile([C, 4 * Q], F16, tag="i2t")
            nc.sync.dma_start(i2t[:], i2_d[:])
            w0m = pool.tile([C, O], F16, tag="w0m")
            nc.sync.dma_start(w0m[:], w0_d[:])
            w1m = pool.tile([C, O], F8, tag="w1m")
            nc.sync.dma_start(w1m[:], w1_d[:])
            gwt = pool.tile([C, 4 * O], F16, tag="gwt")
            nc.sync.dma_start(gwt[:], gwt_d[:])
            pwt = pool.tile([C, 6 * O], F16, tag="pwt")
            nc.sync.dma_start(pwt[:], pwt_d[:])
            biasr = pool.tile([1, O], F32, tag="biasr")
            nc.sync.dma_start(biasr[:], biasr_d[:])
            wsb6 = pool.tile([C, O], F32R, tag="wsb6")
            nc.sync.dma_start(wsb6[:], wsb6_d[:].bitcast(F32R))

            # ---- big tiles: rtA (early-exchange source), rtB (G gate),
            # then fp8 vt partners in main-loop consumption order ----
            rtA = pool.tile([C, QF], F16, tag="rtA")
            rtB = pool.tile([C, QF], F16, tag="rtB")
            vtA = pool.tile([C, QF], F8, tag="vtA")
            vtB = pool.tile([C, QF], F8, tag="vtB")
            # Every tile's chunks alternate between the two HWDGE queues
            # (sync + scalar) so both queues finish rtA+rtB before EITHER
            # starts vt: with per-tensor queue assignment the sync queue
            # ran ahead into vt and stole HBM bandwidth from the rtB tail.
            # Exchange DMAs ride the gpsimd SWDGE queue (a gated entry at
            # a HWDGE queue head would block everything behind it).
            for rt, src in ((rtA, rtA_d), (rtB, rtB_d)):
                for k in range(NCHUNK):
                    sl = slice(k * CH, (k + 1) * CH)
                    nc.sync.dma_start(rt[:, sl], src[:, sl])
            for rt, src in ((vtB, vtB_d), (vtA, vtA_d)):
                for k in range(NVCHUNK):
                    sl = slice(k * VCH, (k + 1) * VCH)
                    nc.sync.dma_start(rt[:, sl], src[:, sl])

            ident = i2t[:, 0:Q]
            # f16 marginal packs [rs | cs]: DVE reduces hit 2x_1p mode and
            # the exchange ships without a cast hop. Marginal magnitudes
            # are ~16-64; f16 rounding there is invisible vs the 2e-2 gate.
            pack1 = pool.tile([C, 256], F16, tag="pack1")  # rs|cs of rtA
            pack2 = pool.tile([C, 256], F16, tag="pack2")  # rs|cs of rtB

            # Rowsums per rt chunk: two f16 tensor_tensor fold levels (DVE
            # 2x_1p mode; InstTensorReduce has NO fast mode) + a small 1x
            # reduce of the 32-wide remainder. 2.9us/chunk vs 4.6 direct.
            # Rowsum fold tree, all-f16 tensor_tensor levels (DVE 2x_1p;
            # both InstTensorReduce and wide reduces run at ~1x) down to
            # 8-wide, then one small reduce. ~1.5us/chunk vs 2.3 direct.
            rows = CH // Q  # 16 rows per chunk
            rs_sc = [pool.tile([C, rows * w], F16, tag=f"rs_s{w}",
                               name=f"rs_s{w}")
                     for w in (64, 32, 16, 8)]

            def rowsums(rt, pack_t):
                s = [t[:].rearrange("p (i j) -> p i j", i=rows, j=w)
                     for t, w in zip(rs_sc, (64, 32, 16, 8))]
                for k in range(NCHUNK):
                    v = rt[:, k * CH:(k + 1) * CH].rearrange(
                        "p (i j) -> p i j", i=rows, j=Q)
                    nc.vector.tensor_tensor(s[0], v[:, :, 0:64],
                                            v[:, :, 64:128], op=ADD)
                    for lv, w in ((1, 32), (2, 16), (3, 8)):
                        nc.vector.tensor_tensor(
                            s[lv], s[lv - 1][:, :, 0:w],
                            s[lv - 1][:, :, w:2 * w], op=ADD)
                    nc.vector.tensor_reduce(
                        pack_t[:, rows * k:rows * (k + 1)], s[3],
                        axis=mybir.AxisListType.X, op=ADD)

            def colsum_fold(ps, pack_t, name):
                # psum->sbuf copy on ACT (a DVE psum copy measured 3.8us;
                # ACT does it in ~0.8 and is idle pre-main)
                r0 = pool.tile([C, 256], F32, tag=f"csc{name}")
                nc.scalar.activation(r0[:], ps[:, 0:256],
                                     mybir.ActivationFunctionType.Copy)
                r1 = pool.tile([C, 256], F32, tag=f"csr{name}")
                nc.vector.tensor_tensor(r1[:], r0[:], ps[:, 256:512], op=ADD)
                nc.vector.tensor_tensor(pack_t[:, Q:2 * Q],
                                        r1[:, 0:Q], r1[:, Q:2 * Q], op=ADD)

            def colsum_mm(rt, name):
                ps = ppaux.tile([C, 512], F32, tag="aux", name=f"cs{name}")
                for m in range(NG):
                    nc.tensor.matmul(ps[:], ident,
                                     rt[:, m * 512:(m + 1) * 512],
                                     start=(m == 0), stop=(m == NG - 1))
                return ps

            with nc.allow_low_precision("f16 marginal packs; magnitudes ~16"
                                        " and output norm dominated by sa"):
                csA = colsum_mm(rtA, "A")
                rowsums(rtA, pack1)
                colsum_fold(csA, pack1, "A")

                # ---- exchange 1 (rs|cs of rtA) as soon as rtA lands ----
                cc1_in = dram.tile([C, 256], F16, tag="cc1_in")
                cc1_out = dram.tile([2 * C, 256], F16, tag="cc1_out")
                nc.scalar.dma_start(cc1_in[:], pack1[:])
                if use_collective:
                    nc.gpsimd.collective_compute(
                        "AllGather", mybir.AluOpType.bypass,
                        replica_groups=[[0, 1], [2, 3], [4, 5], [6, 7]],
                        ins=[cc1_in[:].opt()], outs=[cc1_out[:].opt()])
                else:
                    nc.scalar.dma_start(cc1_out[0:C, :], cc1_in[:])
                    nc.scalar.dma_start(cc1_out[C:2 * C, :], cc1_in[:])
                ex1r0 = pool.tile([C, 256], F16, tag="ex1r0")
                nc.scalar.dma_start(ex1r0[:], cc1_out[0:C, :])
                ex1r1 = pool.tile([C, 256], F16, tag="ex1r1")
                nc.scalar.dma_start(ex1r1[:], cc1_out[C:2 * C, :])

                # ---- PE cover work during the rtB window ----
                parked = []
                for g in range(PARK):
                    ps = ppmain.tile([O, 512], F32, tag="main", name=f"mB{g}")
                    nc.tensor.matmul(ps[:], w0m[:],
                                     rtB[:, g * 512:(g + 1) * 512],
                                     start=True, stop=False)
                    parked.append(ps)
                ps_dummy = ppaux.tile([C, 512], F32, tag="aux", name="dummy")
                for k in range(12):
                    nc.tensor.matmul(ps_dummy[:], ident,
                                     rtA[:, k * 512:(k + 1) * 512],
                                     start=(k == 0), stop=(k == 11))

                csB = colsum_mm(rtB, "B")
                rowsums(rtB, pack2)
                colsum_fold(csB, pack2, "B")
                # Cm_my = my cs(rtA) + my cs(rtB): full column-block-h sums
                cm16 = pool.tile([C, Q], F16, tag="cm16")
                nc.vector.tensor_tensor(cm16[:], pack1[:, Q:2 * Q],
                                        pack2[:, Q:2 * Q], op=ADD)

                # short keep-warm until G (parked W1 happens in the main
                # loop: vtB chunk 0 lands right at G-time, and a pre-G
                # parked-W1 would stall the in-order PE queue on it)
                for k in range(3):
                    nc.tensor.matmul(ps_dummy[:], ident,
                                     rtA[:, (12 + k) * 512:(13 + k) * 512],
                                     start=(k == 0), stop=(k == 2))

                # ---- exchange 2 (rs|cs of rtB): P_rtA + sa only ----
                cc2_in = dram.tile([C, 256], F16, tag="cc2_in")
                cc2_out = dram.tile([2 * C, 256], F16, tag="cc2_out")
                nc.scalar.dma_start(cc2_in[:], pack2[:])
                if use_collective:
                    nc.gpsimd.collective_compute(
                        "AllGather", mybir.AluOpType.bypass,
                        replica_groups=[[0, 1], [2, 3], [4, 5], [6, 7]],
                        ins=[cc2_in[:].opt()], outs=[cc2_out[:].opt()])
                else:
                    nc.scalar.dma_start(cc2_out[0:C, :], cc2_in[:])
                    nc.scalar.dma_start(cc2_out[C:2 * C, :], cc2_in[:])
                ex2r0 = pool.tile([C, 256], F16, tag="ex2r0")
                nc.scalar.dma_start(ex2r0[:], cc2_out[0:C, :])
                ex2r1 = pool.tile([C, 256], F16, tag="ex2r1")
                nc.scalar.dma_start(ex2r1[:], cc2_out[C:2 * C, :])

            # ---- G tile [j, o] (shared by both quads; gates the mains) ----
            # G = R-block-h@W4 + Cm-block-h@W3; R-block-h = my rs(rtB)
            # + sibling rs(rtA) (ex1; rank slots host-zeroed in gwt).
            ps_g = ppaux.tile([Q, O], F32, tag="aux", name="psg")
            nc.tensor.matmul(ps_g[:], pack2[:, 0:Q], gwt[:, 0:O],
                             start=True, stop=False)
            nc.tensor.matmul(ps_g[:], ex1r0[:, 0:Q], gwt[:, O:2 * O],
                             start=False, stop=False)
            nc.tensor.matmul(ps_g[:], ex1r1[:, 0:Q], gwt[:, 2 * O:3 * O],
                             start=False, stop=False)
            nc.tensor.matmul(ps_g[:], cm16[:], gwt[:, 3 * O:4 * O],
                             start=False, stop=True)
            gt = pool.tile([Q, O], F16, tag="gt")
            nc.vector.tensor_copy(gt[:], ps_g[:])

            # ---- P_rtB [o, i in block h] (drains of first quad) ----
            ps_pb = ppaux.tile([O, Q], F32, tag="aux", name="pspb")
            nc.tensor.matmul(ps_pb[:], pwt[:, 0:O], pack2[:, 0:Q],
                             start=True, stop=False)
            nc.tensor.matmul(ps_pb[:], pwt[:, O:2 * O], ex1r0[:, 0:Q],
                             start=False, stop=False)
            nc.tensor.matmul(ps_pb[:], pwt[:, 2 * O:3 * O], ex1r1[:, 0:Q],
                             start=False, stop=False)
            nc.tensor.matmul(ps_pb[:], pwt[:, 3 * O:4 * O], cm16[:],
                             start=False, stop=True)
            pt_b = pool.tile([O, Q], F32, tag="pt_b")
            nc.vector.tensor_copy(pt_b[:], ps_pb[:])

            # ---- main loop quad 1: rtB = (h, h) ----
            dr_idx = 0

            GPS = STB // 512  # psum groups per staging buffer

            def quad_mains(rt_self, vt_self, pt, out_t, qn, parked):
                nonlocal dr_idx
                for blk in range(NG // GPS):
                    stage = stpool.tile([O, STB], F8, tag="stage",
                                        name=f"st{qn}{blk}")
                    for sub in range(GPS):
                        g = blk * GPS + sub
                        sl = slice(g * 512, (g + 1) * 512)
                        if parked is not None and g < PARK:
                            ps = parked[g]
                            nc.tensor.matmul(ps[:], w1m[:], vt_self[:, sl],
                                             start=False, stop=False)
                        else:
                            ps = ppmain.tile([O, 512], F32, tag="main",
                                             name=f"m{qn}{g}")
                            nc.tensor.matmul(ps[:], w0m[:], rt_self[:, sl],
                                             start=True, stop=False)
                            nc.tensor.matmul(ps[:], w1m[:], vt_self[:, sl],
                                             start=False, stop=False)
                        nc.tensor.matmul(ps[:], gt[:], i2t[:],
                                         start=False, stop=True)
                        use_dve = _drain_on_dve(dr_idx)
                        dr_idx += 1
                        off = sub * 512
                        if use_dve:
                            pb = pt[:, 4 * g:4 * g + 4].unsqueeze(2) \
                                .broadcast_to([O, 4, Q])
                            nc.vector.tensor_tensor(
                                stage[:, off:off + 512], ps[:], pb, op=ADD)
                        else:
                            for r in range(4):
                                i_loc = 4 * g + r
                                src = ps[:, r * Q:(r + 1) * Q]
                                dst = stage[:, off + r * Q:off + (r + 1) * Q]
                                nc.scalar.activation(
                                    dst, src,
                                    mybir.ActivationFunctionType.Identity,
                                    bias=pt[:, i_loc:i_loc + 1], scale=1.0)
                    nc.sync.dma_start(out_t[:, blk * STB:(blk + 1) * STB],
                                      stage[:])

            quad_mains(rtB, vtB, pt_b, outB, "B", parked)

            # ---- P_rtA [o, i in block 1-h] (ex2-gated; placed after the
            # first quad's mains so the PE never stalls on ex2) ----
            ps_pa = ppaux.tile([O, Q], F32, tag="aux", name="pspa")
            nc.tensor.matmul(ps_pa[:], pwt[:, 0:O], pack1[:, 0:Q],
                             start=True, stop=False)
            nc.tensor.matmul(ps_pa[:], pwt[:, O:2 * O], ex2r0[:, 0:Q],
                             start=False, stop=False)
            nc.tensor.matmul(ps_pa[:], pwt[:, 2 * O:3 * O], ex2r1[:, 0:Q],
                             start=False, stop=False)
            nc.tensor.matmul(ps_pa[:], pwt[:, 4 * O:5 * O], ex1r0[:, Q:2 * Q],
                             start=False, stop=False)
            nc.tensor.matmul(ps_pa[:], pwt[:, 5 * O:6 * O], ex1r1[:, Q:2 * Q],
                             start=False, stop=False)
            nc.tensor.matmul(ps_pa[:], pwt[:, 4 * O:5 * O], ex2r0[:, Q:2 * Q],
                             start=False, stop=False)
            nc.tensor.matmul(ps_pa[:], pwt[:, 5 * O:6 * O], ex2r1[:, Q:2 * Q],
                             start=False, stop=True)
            pt_a = pool.tile([O, Q], F32, tag="pt_a")
            nc.vector.tensor_copy(pt_a[:], ps_pa[:])

            # ---- sa / sbrow (to host; added back during unshard) ----
            sa_acc = pool.tile([C, 4], F32, tag="sa_acc")
            for idx, t in enumerate((ex1r0, ex1r1, ex2r0, ex2r1)):
                nc.vector.tensor_reduce(sa_acc[:, idx:idx + 1], t[:],
                                        axis=mybir.AxisListType.X, op=ADD)
            sa2 = pool.tile([C, 1], F32, tag="sa2")
            nc.vector.tensor_reduce(sa2[:], sa_acc[:],
                                    axis=mybir.AxisListType.X, op=ADD)
            saT = pool.tile([C, 1], F32R, tag="saT")
            nc.vector.tensor_copy(saT[:], sa2[:].bitcast(F32R))
            ps_s = ppaux.tile([1, O], F32, tag="aux", name="ps_s")
            nc.tensor.matmul(ps_s[:], saT[:], wsb6[:], start=True, stop=True)
            sbrow = pool.tile([1, O], F32, tag="sbrow")
            nc.vector.tensor_tensor(sbrow[:], biasr[:], ps_s[:], op=ADD)
            nc.scalar.dma_start(sbrow_o[:], sbrow[:])

            # ---- main loop quad 2: rtA = (1-h, h) ----
            quad_mains(rtA, vtA, pt_a, outA, "A", None)
    return nc


def _get_nc(use_collective=True):
    key = use_collective
    if key not in _NC_CACHE:
        nc = build(use_collective)
        nc.finalize()
        _NC_CACHE[key] = nc
    return _NC_CACHE[key]


def _host_prep(E, W, bias):
    """Build per-core in_maps from full inputs (column-pairing)."""
    eye = np.eye(Q, dtype=np.float32)
    i2 = np.concatenate([eye] * 4, axis=1).astype(E_NP)
    biasr = bias.reshape(1, O).astype(np.float32)
    wsb6 = (W[6] * 0.5).astype(np.float32)
    W2, W3, W4, W5 = W[2], W[3], W[4], W[5]
    zero = np.zeros((C, O), np.float32)

    in_maps = []
    for core in range(N_CORES):
        b, h = core // 2, core % 2

        def quad_i(p, q):
            blk = E[b, p * Q:(p + 1) * Q, q * Q:(q + 1) * Q, :]
            return np.ascontiguousarray(
                blk.transpose(2, 0, 1)).reshape(C, QF).astype(E_NP)

        def quad_j(p, q):
            blk = E[b, p * Q:(p + 1) * Q, q * Q:(q + 1) * Q, :]
            return np.ascontiguousarray(
                blk.transpose(2, 1, 0)).reshape(C, QF).astype(E8_NP)

        # sibling occupies AllGather rank 1-h; zero the own-rank slots
        w4r0 = W4 if h == 1 else zero
        w4r1 = W4 if h == 0 else zero
        w2r0 = W2 if h == 1 else zero
        w2r1 = W2 if h == 0 else zero
        w5r0 = W5 if h == 1 else zero
        w5r1 = W5 if h == 0 else zero
        gwt = np.concatenate([W4, w4r0, w4r1, W3], axis=1).astype(E_NP)
        pwt = np.concatenate([W2, w2r0, w2r1, W5, w5r0, w5r1],
                             axis=1).astype(E_NP)

        in_maps.append({
            # rtA = quad (1-h, h): ships first, feeds ex1.
            # rtB = quad (h, h): G gates on its marginals.
            "rtA": quad_i(1 - h, h), "rtB": quad_i(h, h),
            # out-quad (p, q) pairs with quad (q, p) j-major
            "vtA": quad_j(h, 1 - h), "vtB": quad_j(h, h),
            "w0m": W[0].astype(E_NP), "w1m": W[1].astype(E8_NP),
            "i2": i2, "gwt": gwt, "pwt": pwt,
            "wsb6": wsb6, "biasr": biasr,
        })
    return in_maps


def _unshard(results, dtype):
    out = np.empty((B, N, N, O), dtype=dtype)
    for core in range(N_CORES):
        b, h = core // 2, core % 2
        sbrow = results[core]["sbrow_o"].astype(np.float32).reshape(O)
        for name, (p, q) in (("outA", (1 - h, h)), ("outB", (h, h))):
            arr = results[core][name].astype(np.float32).reshape(O, Q, Q)
            out[b, p * Q:(p + 1) * Q, q * Q:(q + 1) * Q, :] = \
                arr.transpose(1, 2, 0) + sbrow[None, None, :]
    return out


def kernel(x=None, adj=None, edge_attrs=None, W=None, bias=None, **_):
    E = np.asarray(edge_attrs, dtype=np.float32)
    Wf = np.asarray(W, dtype=np.float32)
    bf = np.asarray(bias, dtype=np.float32)
    in_maps = _host_prep(E, Wf, bf)
    nc = _get_nc(use_collective=True)
    res = run_bass_kernel_spmd(nc, in_maps, core_ids=list(range(N_CORES)))
    return _unshard(res.results, np.float32)


# revision 23
# speedup vs baseline: 1.0263x; 1.0263x over previous
"""Trainium2 Bass kernel (b4) for nn_DenseGraphConvEdgeToEdge (B=4, N=256, C=O=128).

out[b,i,j,:] = E[b,i,j]@W0 + E[b,j,i]@W1 + R[b,i]@W2 + Cm[b,j]@W3
             + R[b,j]@W4 + Cm[b,i]@W5 + sa[b]@W6 + bias
where R = E.sum(axis=2) (row sums), Cm = E.sum(axis=1) (col sums),
sa = E.sum(axis=(1,2)).

b4 column-pairing restructure (vs b3's 112us): core (b, h) owns the two
quadrants of COLUMN h of batch b: rtA = quad (1-h, h) ships first,
rtB = quad (h, h) second. Both output quads share one column block, so
the j-dependent broadcast tile G[j] = Cm[j]@W3 + R[j]@W4 is a SINGLE
tile per core, and its inputs are: Cm-block-h (fully local: my two cs),
my rs(rtB), and the sibling's rs(rtA) - which arrives in an EARLY
pairwise AllGather (ex1) fired as soon as rtA lands (~16us). The second
exchange (rs/cs of rtB) is only needed by P_rtA (drain-time of the
second quad) and sa, both hidden under the first quad's mains. Rank
asymmetry (which AllGather slot is the sibling) is handled by
host-zeroed weight variants in gwt/pwt - 128-row dead matmuls instead
of branches. Marginal packs are f16 end-to-end: DVE rowsum reduces run
in 2x_1p mode (all-16-bit operands), and the exchange payload needs no
cast hop. PE does colsums via identity-matmul. DMA: rt fp16, vt/W1
fp8e4, out fp8e4 centered by sa@W6+bias (shipped as a f32 row, added
back on host: per-channel zero-point dequant). Host-measured rel err of
the quantization scheme: 5.4e-3 (gate 2e-2).
"""
import numpy as np
import ml_dtypes

import concourse.mybir as mybir
import concourse.tile as tile
from concourse import bacc
from concourse.bass_utils import run_bass_kernel_spmd

F32 = mybir.dt.float32
F32R = mybir.dt.float32r
F16 = mybir.dt.float16
F8 = mybir.dt.float8e4
ADD = mybir.AluOpType.add
E_NP = np.float16
E8_NP = ml_dtypes.float8_e4m3

B, N, C, O = 4, 256, 128, 128
Q = 128
QF = Q * Q          # 16384
N_CORES = 8
NCHUNK = 8          # rt chunks (512KB; 4KB/partition descriptors)
CH = QF // NCHUNK   # 2048 cols per rt DMA chunk
NVCHUNK = 2         # fp8 vt chunks: 8KB/partition descriptors
VCH = QF // NVCHUNK
NG = 32             # 512-col psum groups per quadrant
STB = 4096          # staging width per output DMA (4KB/partition fp8)
PARK = 5
DVE_SHARE = 42      # of 64 drains on DVE (512-wide broadcast t_t), rest ACT

_NC_CACHE = {}


def _drain_on_dve(idx, total=2 * NG, share=DVE_SHARE):
    return (idx * share) // total != ((idx + 1) * share) // total


def build(use_collective=True):
    nc = bacc.Bacc(trn_type="TRN2")

    rtA_d = nc.dram_tensor("rtA", [C, QF], F16, kind="ExternalInput")
    rtB_d = nc.dram_tensor("rtB", [C, QF], F16, kind="ExternalInput")
    vtA_d = nc.dram_tensor("vtA", [C, QF], F8, kind="ExternalInput")
    vtB_d = nc.dram_tensor("vtB", [C, QF], F8, kind="ExternalInput")
    w0_d = nc.dram_tensor("w0m", [C, O], F16, kind="ExternalInput")
    w1_d = nc.dram_tensor("w1m", [C, O], F8, kind="ExternalInput")
    i2_d = nc.dram_tensor("i2", [C, 4 * Q], F16, kind="ExternalInput")
    gwt_d = nc.dram_tensor("gwt", [C, 4 * O], F16, kind="ExternalInput")
    pwt_d = nc.dram_tensor("pwt", [C, 6 * O], F16, kind="ExternalInput")
    wsb6_d = nc.dram_tensor("wsb6", [C, O], F32, kind="ExternalInput")
    biasr_d = nc.dram_tensor("biasr", [1, O], F32, kind="ExternalInput")
    outA = nc.dram_tensor("outA", [O, QF], F8, kind="ExternalOutput")
    outB = nc.dram_tensor("outB", [O, QF], F8, kind="ExternalOutput")
    sbrow_o = nc.dram_tensor("sbrow_o", [1, O], F32, kind="ExternalOutput")

    with tile.TileContext(nc) as tc:
        with (
            tc.tile_pool(name="pool", bufs=1) as pool,
            tc.tile_pool(name="stpool", bufs=3) as stpool,
            tc.tile_pool(name="ppmain", bufs=5, space="PSUM") as ppmain,
            tc.tile_pool(name="ppaux", bufs=3, space="PSUM") as ppaux,
            tc.tile_pool(name="dram", bufs=1, space="DRAM") as dram,
        ):
            # ---- small constants first ----
            i2t = pool.tile([C, 4 * Q], F16, tag="i2t")
            nc.sync.dma_start(i2t[:], i2_d[:])
            w0m = pool.tile([C, O], F16, tag="w0m")
            nc.sync.dma_start(w0m[:], w0_d[:])
            w1m = pool.tile([C, O], F8, tag="w1m")
            nc.sync.dma_start(w1m[:], w1_d[:])
            gwt = pool.tile([C, 4 * O], F16, tag="gwt")
            nc.sync.dma_start(gwt[:], gwt_d[:])
            pwt = pool.tile([C, 6 * O], F16, tag="pwt")
            nc.sync.dma_start(pwt[:], pwt_d[:])
            biasr = pool.tile([1, O], F32, tag="biasr")
            nc.sync.dma_start(biasr[:], biasr_d[:])
            wsb6 = pool.tile([C, O], F32R, tag="wsb6")
            nc.sync.dma_start(wsb6[:], wsb6_d[:].bitcast(F32R))

            # ---- big tiles: rtA (early-exchange source), rtB (G gate),
            # then fp8 vt partners in main-loop consumption order ----
            rtA = pool.tile([C, QF], F16, tag="rtA")
            rtB = pool.tile([C, QF], F16, tag="rtB")
            vtA = pool.tile([C, QF], F8, tag="vtA")
            vtB = pool.tile([C, QF], F8, tag="vtB")
            # Every tile's chunks alternate between the two HWDGE queues
            # (sync + scalar) so both queues finish rtA+rtB before EITHER
            # starts vt: with per-tensor queue assignment the sync queue
            # ran ahead into vt and stole HBM bandwidth from the rtB tail.
            # Exchange DMAs ride the gpsimd SWDGE queue (a gated entry at
            # a HWDGE queue head would block everything behind it).
            for rt, src in ((rtA, rtA_d), (rtB, rtB_d)):
                for k in range(NCHUNK - 1):
                    sl = slice(k * CH, (k + 1) * CH)
                    eng = nc.sync if k % 2 == 0 else nc.scalar
                    eng.dma_start(rt[:, sl], src[:, sl])
                # split the gating last chunk across both queues so its
                # final bytes (and the DVE tail trio) land ~1us earlier
                lo = (NCHUNK - 1) * CH
                nc.scalar.dma_start(rt[:, lo:lo + CH // 2],
                                    src[:, lo:lo + CH // 2])
                nc.sync.dma_start(rt[:, lo + CH // 2:lo + CH],
                                  src[:, lo + CH // 2:lo + CH])
            for rt, src in ((vtB, vtB_d), (vtA, vtA_d)):
                for k in range(NVCHUNK):
                    sl = slice(k * VCH, (k + 1) * VCH)
                    nc.sync.dma_start(rt[:, sl], src[:, sl])

            ident = i2t[:, 0:Q]
            # f16 marginal packs [rs | cs]: DVE reduces hit 2x_1p mode and
            # the exchange ships without a cast hop. Marginal magnitudes
            # are ~16-64; f16 rounding there is invisible vs the 2e-2 gate.
            pack1 = pool.tile([C, 256], F16, tag="pack1")  # rs|cs of rtA
            pack2 = pool.tile([C, 256], F16, tag="pack2")  # rs|cs of rtB

            # Rowsums per rt chunk: two f16 tensor_tensor fold levels (DVE
            # 2x_1p mode; InstTensorReduce has NO fast mode) + a small 1x
            # reduce of the 32-wide remainder. 2.9us/chunk vs 4.6 direct.
            # Rowsum fold tree, all-f16 tensor_tensor levels (DVE 2x_1p;
            # both InstTensorReduce and wide reduces run at ~1x) down to
            # 8-wide, then one small reduce. ~1.5us/chunk vs 2.3 direct.
            rows = CH // Q  # 16 rows per chunk
            rs_sc = [pool.tile([C, rows * w], F16, tag=f"rs_s{w}",
                               name=f"rs_s{w}")
                     for w in (64, 32, 16, 8)]

            def rowsums(rt, pack_t):
                s = [t[:].rearrange("p (i j) -> p i j", i=rows, j=w)
                     for t, w in zip(rs_sc, (64, 32, 16, 8))]
                for k in range(NCHUNK):
                    v = rt[:, k * CH:(k + 1) * CH].rearrange(
                        "p (i j) -> p i j", i=rows, j=Q)
                    nc.vector.tensor_tensor(s[0], v[:, :, 0:64],
                                            v[:, :, 64:128], op=ADD)
                    for lv, w in ((1, 32), (2, 16), (3, 8)):
                        nc.vector.tensor_tensor(
                            s[lv], s[lv - 1][:, :, 0:w],
                            s[lv - 1][:, :, w:2 * w], op=ADD)
                    nc.vector.tensor_reduce(
                        pack_t[:, rows * k:rows * (k + 1)], s[3],
                        axis=mybir.AxisListType.X, op=ADD)

            def colsum_fold(ps, pack_t, name):
                # psum->sbuf copy on ACT (a DVE psum copy measured 3.8us;
                # ACT does it in ~0.8 and is idle pre-main)
                r0 = pool.tile([C, 256], F32, tag=f"csc{name}")
                nc.scalar.activation(r0[:], ps[:, 0:256],
                                     mybir.ActivationFunctionType.Copy)
                r1 = pool.tile([C, 256], F32, tag=f"csr{name}")
                nc.vector.tensor_tensor(r1[:], r0[:], ps[:, 256:512], op=ADD)
                nc.vector.tensor_tensor(pack_t[:, Q:2 * Q],
                                        r1[:, 0:Q], r1[:, Q:2 * Q], op=ADD)

            def colsum_mm(rt, name):
                ps = ppaux.tile([C, 512], F32, tag="aux", name=f"cs{name}")
                for m in range(NG):
                    nc.tensor.matmul(ps[:], ident,
                                     rt[:, m * 512:(m + 1) * 512],
                                     start=(m == 0), stop=(m == NG - 1))
                return ps

            with nc.allow_low_precision("f16 marginal packs; magnitudes ~16"
                                        " and output norm dominated by sa"):
                csA = colsum_mm(rtA, "A")
                rowsums(rtA, pack1)
                colsum_fold(csA, pack1, "A")

                # ---- exchange 1 (rs|cs of rtA) as soon as rtA lands ----
                cc1_in = dram.tile([C, 256], F16, tag="cc1_in")
                cc1_out = dram.tile([2 * C, 256], F16, tag="cc1_out")
                nc.scalar.dma_start(cc1_in[:], pack1[:])
                if use_collective:
                    nc.gpsimd.collective_compute(
                        "AllGather", mybir.AluOpType.bypass,
                        replica_groups=[[0, 1], [2, 3], [4, 5], [6, 7]],
                        ins=[cc1_in[:].opt()], outs=[cc1_out[:].opt()])
                else:
                    nc.scalar.dma_start(cc1_out[0:C, :], cc1_in[:])
                    nc.scalar.dma_start(cc1_out[C:2 * C, :], cc1_in[:])
                ex1r0 = pool.tile([C, 256], F16, tag="ex1r0")
                nc.scalar.dma_start(ex1r0[:], cc1_out[0:C, :])
                ex1r1 = pool.tile([C, 256], F16, tag="ex1r1")
                nc.scalar.dma_start(ex1r1[:], cc1_out[C:2 * C, :])

                # ---- PE cover work during the rtB window ----
                parked = []
                for g in range(PARK):
                    ps = ppmain.tile([O, 512], F32, tag="main", name=f"mB{g}")
                    nc.tensor.matmul(ps[:], w0m[:],
                                     rtB[:, g * 512:(g + 1) * 512],
                                     start=True, stop=False)
                    parked.append(ps)
                ps_dummy = ppaux.tile([C, 512], F32, tag="aux", name="dummy")
                for k in range(12):
                    nc.tensor.matmul(ps_dummy[:], ident,
                                     rtA[:, k * 512:(k + 1) * 512],
                                     start=(k == 0), stop=(k == 11))

                csB = colsum_mm(rtB, "B")
                rowsums(rtB, pack2)
                colsum_fold(csB, pack2, "B")
                # Cm_my = my cs(rtA) + my cs(rtB): full column-block-h sums
                cm16 = pool.tile([C, Q], F16, tag="cm16")
                nc.vector.tensor_tensor(cm16[:], pack1[:, Q:2 * Q],
                                        pack2[:, Q:2 * Q], op=ADD)

                # short keep-warm until G (parked W1 happens in the main
                # loop: vtB chunk 0 lands right at G-time, and a pre-G
                # parked-W1 would stall the in-order PE queue on it)
                for k in range(16):
                    nc.tensor.matmul(ps_dummy[:], ident,
                                     rtA[:, (8 + k) * 512:(9 + k) * 512],
                                     start=(k == 0), stop=(k == 15))

                # ---- exchange 2 (rs|cs of rtB): P_rtA + sa only ----
                cc2_in = dram.tile([C, 256], F16, tag="cc2_in")
                cc2_out = dram.tile([2 * C, 256], F16, tag="cc2_out")
                nc.scalar.dma_start(cc2_in[:], pack2[:])
                if use_collective:
                    nc.gpsimd.collective_compute(
                        "AllGather", mybir.AluOpType.bypass,
                        replica_groups=[[0, 1], [2, 3], [4, 5], [6, 7]],
                        ins=[cc2_in[:].opt()], outs=[cc2_out[:].opt()])
                else:
                    nc.scalar.dma_start(cc2_out[0:C, :], cc2_in[:])
                    nc.scalar.dma_start(cc2_out[C:2 * C, :], cc2_in[:])
                ex2r0 = pool.tile([C, 256], F16, tag="ex2r0")
                nc.scalar.dma_start(ex2r0[:], cc2_out[0:C, :])
                ex2r1 = pool.tile([C, 256], F16, tag="ex2r1")
                nc.scalar.dma_start(ex2r1[:], cc2_out[C:2 * C, :])

            # ---- G tile [j, o] (shared by both quads; gates the mains) ----
            # G = R-block-h@W4 + Cm-block-h@W3; R-block-h = my rs(rtB)
            # + sibling rs(rtA) (ex1; rank slots host-zeroed in gwt).
            ps_g = ppaux.tile([Q, O], F32, tag="aux", name="psg")
            nc.tensor.matmul(ps_g[:], pack2[:, 0:Q], gwt[:, 0:O],
                             start=True, stop=False)
            nc.tensor.matmul(ps_g[:], ex1r0[:, 0:Q], gwt[:, O:2 * O],
                             start=False, stop=False)
            nc.tensor.matmul(ps_g[:], ex1r1[:, 0:Q], gwt[:, 2 * O:3 * O],
                             start=False, stop=False)
            nc.tensor.matmul(ps_g[:], cm16[:], gwt[:, 3 * O:4 * O],
                             start=False, stop=True)
            gt = pool.tile([Q, O], F16, tag="gt")
            nc.vector.tensor_copy(gt[:], ps_g[:])

            # ---- P_rtB [o, i in block h] (drains of first quad) ----
            ps_pb = ppaux.tile([O, Q], F32, tag="aux", name="pspb")
            nc.tensor.matmul(ps_pb[:], pwt[:, 0:O], pack2[:, 0:Q],
                             start=True, stop=False)
            nc.tensor.matmul(ps_pb[:], pwt[:, O:2 * O], ex1r0[:, 0:Q],
                             start=False, stop=False)
            nc.tensor.matmul(ps_pb[:], pwt[:, 2 * O:3 * O], ex1r1[:, 0:Q],
                             start=False, stop=False)
            nc.tensor.matmul(ps_pb[:], pwt[:, 3 * O:4 * O], cm16[:],
                             start=False, stop=True)
            pt_b = pool.tile([O, Q], F32, tag="pt_b")
            nc.vector.tensor_copy(pt_b[:], ps_pb[:])

            # ---- main loop quad 1: rtB = (h, h) ----
            dr_idx = 0

            GPS = STB // 512  # psum groups per staging buffer

            def quad_mains(rt_self, vt_self, pt, out_t, qn, parked):
                nonlocal dr_idx
                for blk in range(NG // GPS):
                    stage = stpool.tile([O, STB], F8, tag="stage",
                                        name=f"st{qn}{blk}")
                    for sub in range(GPS):
                        g = blk * GPS + sub
                        sl = slice(g * 512, (g + 1) * 512)
                        if parked is not None and g < PARK:
                            ps = parked[g]
                            nc.tensor.matmul(ps[:], w1m[:], vt_self[:, sl],
                                             start=False, stop=False)
                        else:
                            ps = ppmain.tile([O, 512], F32, tag="main",
                                             name=f"m{qn}{g}")
                            nc.tensor.matmul(ps[:], w0m[:], rt_self[:, sl],
                                             start=True, stop=False)
                            nc.tensor.matmul(ps[:], w1m[:], vt_self[:, sl],
                                             start=False, stop=False)
                        nc.tensor.matmul(ps[:], gt[:], i2t[:],
                                         start=False, stop=True)
                        use_dve = _drain_on_dve(dr_idx)
                        dr_idx += 1
                        off = sub * 512
                        if use_dve:
                            pb = pt[:, 4 * g:4 * g + 4].unsqueeze(2) \
                                .broadcast_to([O, 4, Q])
                            nc.vector.tensor_tensor(
                                stage[:, off:off + 512], ps[:], pb, op=ADD)
                        else:
                            for r in range(4):
                                i_loc = 4 * g + r
                                src = ps[:, r * Q:(r + 1) * Q]
                                dst = stage[:, off + r * Q:off + (r + 1) * Q]
                                nc.scalar.activation(
                                    dst, src,
                                    mybir.ActivationFunctionType.Identity,
                                    bias=pt[:, i_loc:i_loc + 1], scale=1.0)
                    nc.sync.dma_start(out_t[:, blk * STB:(blk + 1) * STB],
                                      stage[:])

            quad_mains(rtB, vtB, pt_b, outB, "B", parked)

            # ---- P_rtA [o, i in block 1-h] (ex2-gated; placed after the
            # first quad's mains so the PE never stalls on ex2) ----
            ps_pa = ppaux.tile([O, Q], F32, tag="aux", name="pspa")
            nc.tensor.matmul(ps_pa[:], pwt[:, 0:O], pack1[:, 0:Q],
                             start=True, stop=False)
            nc.tensor.matmul(ps_pa[:], pwt[:, O:2 * O], ex2r0[:, 0:Q],
                             start=False, stop=False)
            nc.tensor.matmul(ps_pa[:], pwt[:, 2 * O:3 * O], ex2r1[:, 0:Q],
                             start=False, stop=False)
            nc.tensor.matmul(ps_pa[:], pwt[:, 4 * O:5 * O], ex1r0[:, Q:2 * Q],
                             start=False, stop=False)
            nc.tensor.matmul(ps_pa[:], pwt[:, 5 * O:6 * O], ex1r1[:, Q:2 * Q],
                             start=False, stop=False)
            nc.tensor.matmul(ps_pa[:], pwt[:, 4 * O:5 * O], ex2r0[:, Q:2 * Q],
                             start=False, stop=False)
            nc.tensor.matmul(ps_pa[:], pwt[:, 5 * O:6 * O], ex2r1[:, Q:2 * Q],
                             start=False, stop=True)
            pt_a = pool.tile([O, Q], F32, tag="pt_a")
            nc.vector.tensor_copy(pt_a[:], ps_pa[:])

            # ---- sa / sbrow (to host; added back during unshard) ----
            sa_acc = pool.tile([C, 4], F32, tag="sa_acc")
            for idx, t in enumerate((ex1r0, ex1r1, ex2r0, ex2r1)):
                nc.vector.tensor_reduce(sa_acc[:, idx:idx + 1], t[:],
                                        axis=mybir.AxisListType.X, op=ADD)
            sa2 = pool.tile([C, 1], F32, tag="sa2")
            nc.vector.tensor_reduce(sa2[:], sa_acc[:],
                                    axis=mybir.AxisListType.X, op=ADD)
            saT = pool.tile([C, 1], F32R, tag="saT")
            nc.vector.tensor_copy(saT[:], sa2[:].bitcast(F32R))
            ps_s = ppaux.tile([1, O], F32, tag="aux", name="ps_s")
            nc.tensor.matmul(ps_s[:], saT[:], wsb6[:], start=True, stop=True)
            sbrow = pool.tile([1, O], F32, tag="sbrow")
            nc.vector.tensor_tensor(sbrow[:], biasr[:], ps_s[:], op=ADD)
            nc.scalar.dma_start(sbrow_o[:], sbrow[:])

            # ---- main loop quad 2: rtA = (1-h, h) ----
            quad_mains(rtA, vtA, pt_a, outA, "A", None)
    return nc


def _get_nc(use_collective=True):
    key = use_collective
    if key not in _NC_CACHE:
        nc = build(use_collective)
        nc.finalize()
        _NC_CACHE[key] = nc
    return _NC_CACHE[key]


def _host_prep(E, W, bias):
    """Build per-core in_maps from full inputs (column-pairing)."""
    eye = np.eye(Q, dtype=np.float32)
    i2 = np.concatenate([eye] * 4, axis=1).astype(E_NP)
    biasr = bias.reshape(1, O).astype(np.float32)
    wsb6 = (W[6] * 0.5).astype(np.float32)
    W2, W3, W4, W5 = W[2], W[3], W[4], W[5]
    zero = np.zeros((C, O), np.float32)

    in_maps = []
    for core in range(N_CORES):
        b, h = core // 2, core % 2

        def quad_i(p, q):
            blk = E[b, p * Q:(p + 1) * Q, q * Q:(q + 1) * Q, :]
            return np.ascontiguousarray(
                blk.transpose(2, 0, 1)).reshape(C, QF).astype(E_NP)

        def quad_j(p, q):
            blk = E[b, p * Q:(p + 1) * Q, q * Q:(q + 1) * Q, :]
            return np.ascontiguousarray(
                blk.transpose(2, 1, 0)).reshape(C, QF).astype(E8_NP)

        # sibling occupies AllGather rank 1-h; zero the own-rank slots
        w4r0 = W4 if h == 1 else zero
        w4r1 = W4 if h == 0 else zero
        w2r0 = W2 if h == 1 else zero
        w2r1 = W2 if h == 0 else zero
        w5r0 = W5 if h == 1 else zero
        w5r1 = W5 if h == 0 else zero
        gwt = np.concatenate([W4, w4r0, w4r1, W3], axis=1).astype(E_NP)
        pwt = np.concatenate([W2, w2r0, w2r1, W5, w5r0, w5r1],
                             axis=1).astype(E_NP)

        in_maps.append({
            # rtA = quad (1-h, h): ships first, feeds ex1.
            # rtB = quad (h, h): G gates on its marginals.
            "rtA": quad_i(1 - h, h), "rtB": quad_i(h, h),
            # out-quad (p, q) pairs with quad (q, p) j-major
            "vtA": quad_j(h, 1 - h), "vtB": quad_j(h, h),
            "w0m": W[0].astype(E_NP), "w1m": W[1].astype(E8_NP),
            "i2": i2, "gwt": gwt, "pwt": pwt,
            "wsb6": wsb6, "biasr": biasr,
        })
    return in_maps


def _unshard(results, dtype):
    out = np.empty((B, N, N, O), dtype=dtype)
    for core in range(N_CORES):
        b, h = core // 2, core % 2
        sbrow = results[core]["sbrow_o"].astype(np.float32).reshape(O)
        for name, (p, q) in (("outA", (1 - h, h)), ("outB", (h, h))):
            arr = results[core][name].astype(np.float32).reshape(O, Q, Q)
            out[b, p * Q:(p + 1) * Q, q * Q:(q + 1) * Q, :] = \
                arr.transpose(1, 2, 0) + sbrow[None, None, :]
    return out


def kernel(x=None, adj=None, edge_attrs=None, W=None, bias=None, **_):
    E = np.asarray(edge_attrs, dtype=np.float32)
    Wf = np.asarray(W, dtype=np.float32)
    bf = np.asarray(bias, dtype=np.float32)
    in_maps = _host_prep(E, Wf, bf)
    nc = _get_nc(use_collective=True)
    res = run_bass_kernel_spmd(nc, in_maps, core_ids=list(range(N_CORES)))
    return _unshard(res.results, np.float32)
